# revision 26
# baseline (speedup 1.0000x reference)
"""ConvSelfAttention Trainium2 kernel.

Reference computation (per batch b, with x flattened to [C=128, N=4096]):
    q = wq @ x + bq        [64, N]   (scaled by 1/sqrt(128), folded into wq/bq)
    k = wk @ x + bk        [64, N]
    v = wv @ x + bv        [64, N]
    s[i,j] = sum_o q[o,i] k[o,j]
    p = softmax_j(s)
    out[o,i] = sum_j v[o,j] p[i,j]
    y = gamma * (wo @ out + bo) + x

Mapping (one batch per NeuronCore, 8 cores):
  - scores are built TRANSPOSED: sT[j,i] = sum_o k[o,j] q[o,i], j-tile (128) on
    partitions, i-block (512) on free dim; exp runs on ScalarE over 3-bank PSUM
    groups.
  - QK has K=64 (head dim), so q/k are kept DUPLICATED in both partition
    halves (duplication is free: the projection weight matrix is duplicated on
    the host) and consecutive j-tiles run CONCURRENTLY in the PE array via
    row tile_position (0,0)/(64,0).
  - PV uses the exp output pT as the STATIONARY operand (M=128 i-columns, FWL
    loads bf16 weights 2/cycle) streaming the ones-augmented V^T (N=65):
    psum[i, 0:64] accumulates attention output (transposed), psum[i, 64] the
    softmax denominator -- per-partition, so normalization is a plain
    tensor_scalar multiply and the reciprocal is a [128, 4] op.
  - the normalized [i, o] tile is PE-transposed back to [o, i] for the output
    projection; v's bias folds into the constant gamma*(wo@bv+bo) because
    sum_j p = 1; gamma folds into wo/bo on the host.

All matmuls run in bf16 (fp32 PSUM accumulation); exp runs on ScalarE in fp32.
"""

import sys

import numpy as np

try:
    import concourse  # noqa: F401
except ImportError:  # pragma: no cover
    sys.path.insert(0, "/opt/trn_rl_repo")

import ml_dtypes

B, C, CO, N = 8, 128, 64, 4096
W = H = 64
NCORES = 8
IBLK = 512          # query columns per i-block
NSUB = IBLK // 128  # 4 i-subtiles per i-block (PV stationary operand width)
NJT = N // 128      # 32 j-tiles of 128 keys
NIB = N // IBLK     # 8 i-blocks
JGRP = 3            # j-tiles per exp group (3 PSUM banks)

_CACHE = {}


def _split_multiwaits(nc):
    """Workaround for the pinned walrus: it accepts at most ONE semaphore wait
    per instruction (setupSyncWait: "Too many sync wait commands").  Hoist all
    but the last wait of any instruction onto single-wait NoOps inserted just
    before it in the same engine's stream — semantically identical (the engine
    blocks on each wait in turn before issuing the instruction)."""
    from concourse import mybir

    nsplit = 0
    for fn in nc.m.functions:
        for bb in fn.blocks:
            out = []
            for inst in bb.instructions:
                si = inst.sync_info
                if si is not None and si.on_wait is not None and len(si.on_wait) > 1:
                    waits = list(si.on_wait)
                    for i, w in enumerate(waits[:-1]):
                        out.append(mybir.InstNoOp(
                            name=f"{inst.name}-sw{i}",
                            engine=inst.engine,
                            sync_info=mybir.SyncInfo(on_wait=[w], on_update=[]),
                            bass_nofuse=True,
                        ))
                        nsplit += 1
                    si.on_wait = [waits[-1]]
                    inst.sync_info = si
                out.append(inst)
            bb.instructions = out
    return nsplit


def build_nc(debug=False, nib=NIB, use_bacc=False, split=True):
    from concourse import bacc, mybir
    import concourse.bass as bass
    import concourse.tile as tile
    from concourse.masks import make_identity

    f32 = mybir.dt.float32
    bf16 = mybir.dt.bfloat16
    Alu = mybir.AluOpType
    Act = mybir.ActivationFunctionType

    if use_bacc:
        nc = bacc.Bacc(None, target_bir_lowering=False, debug=debug)
    else:
        nc = bass.Bass()

    x_d = nc.dram_tensor("x", [C, N], f32, kind="ExternalInput")
    xb_d = nc.dram_tensor("xb", [C, N], bf16, kind="ExternalInput")  # host cast
    # packed bf16 weights: [wqT(128) | wkT(128) | wvT(64) | woT(128, rows 0:64)]
    wpack_d = nc.dram_tensor("wpack", [C, 448], bf16, kind="ExternalInput")
    # packed f32 scalars: [bq | bk | gbo]
    bpack_d = nc.dram_tensor("bpack", [C, 3], f32, kind="ExternalInput")
    y_d = nc.dram_tensor("y", [C, N], f32, kind="ExternalOutput")

    with tile.TileContext(nc) as tc:
        with (
            tc.tile_pool(name="consts", bufs=1) as consts,
            tc.tile_pool(name="big", bufs=1) as big,
            tc.tile_pool(name="pt", bufs=4) as pt_pool,
            tc.tile_pool(name="epi", bufs=2) as epi,
        ):
            # ---- x chunk 0 DMA first (critical path), then packed weights ----
            x_sb = big.tile([C, N], f32)
            x_bf = big.tile([C, N], bf16)
            nc.sync.dma_start(x_bf[:, 0:512], xb_d[:, 0:512])

            wpack = consts.tile([C, 448], bf16)
            nc.gpsimd.dma_start(wpack, wpack_d[:, :])
            bpack = consts.tile([C, 3], f32)
            nc.gpsimd.dma_start(bpack, bpack_d[:, :])
            wqT = wpack[:, 0:128]
            wkT = wpack[:, 128:256]
            wvT = wpack[:, 256:320]
            woT = wpack[0:CO, 320:448]
            bq_s = bpack[:, 0:1]
            bk_s = bpack[:, 1:2]
            gbo = bpack[:, 2:3]
            ident = consts.tile([C, C], bf16)
            make_identity(nc, ident)

            # warm the exp table set (~2.7us ACT_TABLE_LOAD) during the ramp;
            # memset source so no DMA dependency
            warm = consts.tile([C, 1], f32)
            nc.vector.memset(warm, 0.0)
            nc.scalar.activation(warm, warm, Act.Exp)

            # ---- x load + cast + projections, pipelined in 512-col chunks ----
            q_sb = big.tile([C, N], bf16)
            k_sb = big.tile([C, N], bf16)
            vT = big.tile([C, NJT * (CO + 1)], bf16)  # 32 x [128, 65] tiles
            vT3 = vT.rearrange("p (t e) -> p t e", e=CO + 1)
            nc.vector.memset(vT3[:, :, CO:CO + 1], 1.0)

            with tc.tile_pool(name="setup_ps", bufs=2, space="PSUM") as setup_ps:
                for t in range(N // 512):
                    sl = slice(t * 512, (t + 1) * 512)
                    if t > 0:  # chunk 0 DMA already issued above
                        nc.sync.dma_start(x_bf[:, sl], xb_d[:, sl])
                    # x f32 (residual add) loads independently, off the
                    # projection critical path
                    nc.sync.dma_start(x_sb[:, sl], x_d[:, sl])
                    ps_q = setup_ps.tile([C, 512], f32, tag="proj")
                    nc.tensor.matmul(ps_q, lhsT=wqT, rhs=x_bf[:, sl],
                                     start=True, stop=True)
                    nc.vector.tensor_scalar_add(q_sb[:, sl], ps_q, bq_s)
                    ps_k = setup_ps.tile([C, 512], f32, tag="proj")
                    nc.tensor.matmul(ps_k, lhsT=wkT, rhs=x_bf[:, sl],
                                     start=True, stop=True)
                    nc.vector.tensor_scalar_add(k_sb[:, sl], ps_k, bk_s)
                    ps_v = setup_ps.tile([C, 256], f32, tag="vt")
                    for tt in range(4):
                        nt = t * 4 + tt
                        nc.tensor.matmul(
                            ps_v[:, tt * CO:(tt + 1) * CO],
                            lhsT=x_bf[:, nt * 128:(nt + 1) * 128],
                            rhs=wvT,
                            start=True, stop=True,
                        )
                    nc.vector.tensor_copy(
                        vT3[:, t * 4:(t + 1) * 4, 0:CO],
                        ps_v.rearrange("p (t e) -> p t e", e=CO),
                    )

            # ---- main loop over query blocks ----
            with (
                tc.tile_pool(name="qk_ps", bufs=2, space="PSUM") as qk_ps_pool,
                tc.tile_pool(name="aux_ps", bufs=2, space="PSUM") as aux_ps_pool,
            ):
                jgroups = [list(range(s, min(s + JGRP, NJT)))
                           for s in range(0, NJT, JGRP)]
                for ib in range(nib):
                    isl = slice(ib * IBLK, (ib + 1) * IBLK)
                    # [i, o|denom] accumulators for 4 i-subtiles in ONE bank
                    ps_pv = aux_ps_pool.tile([C, NSUB * (CO + 1)], f32, tag="aux")
                    for jts in jgroups:
                        glen = len(jts)
                        ps_qk = qk_ps_pool.tile([128, JGRP * 512], f32)
                        for idx, jt in enumerate(jts):
                            half = jt % 2  # alternate row halves -> PE pairs
                            hsl = slice(half * CO, half * CO + CO)
                            nc.tensor.matmul(
                                ps_qk[:, idx * 512:(idx + 1) * 512],
                                lhsT=k_sb[hsl, jt * 128:(jt + 1) * 128],
                                rhs=q_sb[hsl, isl],
                                start=True, stop=True,
                            )
                        pT = pt_pool.tile([128, JGRP * 512], bf16)
                        nc.scalar.activation(pT[:, 0:glen * 512],
                                             ps_qk[:, 0:glen * 512], Act.Exp)
                        for idx, jt in enumerate(jts):
                            for s in range(NSUB):
                                # start/stop once per BANK: start=True clears
                                # the whole 2KB zero region, so only the very
                                # first MM starts and the very last stops; the
                                # other sub-slices rely on per-element
                                # has_written (cleared by the start) for
                                # overwrite-then-accumulate semantics.
                                nc.tensor.matmul(
                                    ps_pv[:, s * (CO + 1):(s + 1) * (CO + 1)],
                                    lhsT=pT[:, idx * 512 + s * 128:
                                            idx * 512 + (s + 1) * 128],
                                    rhs=vT3[:, jt, :],
                                    start=(jt == 0 and s == 0),
                                    stop=(jt == NJT - 1 and s == NSUB - 1),
                                )

                    # epilogue: per-partition normalize, transpose, project
                    rden = epi.tile([C, NSUB], f32, tag="rden")
                    pv3 = ps_pv.rearrange("p (s e) -> p s e", e=CO + 1)
                    nc.vector.reciprocal(rden, pv3[:, :, CO])
                    onT = epi.tile([C, NSUB * CO], bf16, tag="onT")
                    for s in range(NSUB):
                        nc.vector.tensor_scalar_mul(
                            onT[:, s * CO:(s + 1) * CO],
                            pv3[:, s, 0:CO],
                            rden[:, s:s + 1],
                        )
                    out_sb = epi.tile([CO, IBLK], bf16, tag="out")
                    for s in range(NSUB):
                        ps_t = aux_ps_pool.tile([CO, 128], bf16, tag="aux")
                        nc.tensor.transpose(ps_t, onT[:, s * CO:(s + 1) * CO],
                                            ident)
                        nc.vector.tensor_copy(out_sb[:, s * 128:(s + 1) * 128],
                                              ps_t)
                    ps_oc = aux_ps_pool.tile([C, IBLK], f32, tag="aux")
                    nc.tensor.matmul(ps_oc, lhsT=woT, rhs=out_sb,
                                     start=True, stop=True)
                    y2 = epi.tile([C, IBLK], f32, tag="y2")
                    nc.vector.scalar_tensor_tensor(
                        out=y2, in0=ps_oc, scalar=gbo, in1=x_sb[:, isl],
                        op0=Alu.add, op1=Alu.add,
                    )
                    nc.sync.dma_start(y_d[:, isl], y2)

    if split:
        _split_multiwaits(nc)
    return nc


def host_prep(inputs):
    """Fold scales/transposes on the host; returns the 8 per-core input maps."""
    x = np.ascontiguousarray(np.asarray(inputs["x"], dtype=np.float32))
    wq = np.asarray(inputs["wq"], dtype=np.float32)
    bq = np.asarray(inputs["bq"], dtype=np.float32)
    wk = np.asarray(inputs["wk"], dtype=np.float32)
    bk = np.asarray(inputs["bk"], dtype=np.float32)
    wv = np.asarray(inputs["wv"], dtype=np.float32)
    bv = np.asarray(inputs["bv"], dtype=np.float32)
    wo = np.asarray(inputs["wo"], dtype=np.float32)
    bo = np.asarray(inputs["bo"], dtype=np.float32)
    gamma = float(np.asarray(inputs["gamma"]).reshape(-1)[0])

    s = 1.0 / np.sqrt(np.float32(C))
    bf = ml_dtypes.bfloat16
    wqTs = wq.T * s                                                    # [128,64]
    wqT = np.concatenate([wqTs, wqTs], axis=1)                         # [128,128]
    wkT = np.concatenate([wk.T, wk.T], axis=1)                         # [128,128]
    wvT = wv.T                                                         # [128,64]
    woT_pad = np.zeros((C, C), np.float32)
    woT_pad[:CO, :] = gamma * wo.T                                     # rows 0:64
    wpack = np.concatenate([wqT, wkT, wvT, woT_pad], axis=1).astype(bf)
    bq_s = np.concatenate([bq * s, bq * s])
    bk_s = np.concatenate([bk, bk])
    gbo = gamma * (wo @ bv + bo)
    bpack = np.stack([bq_s, bk_s, gbo], axis=1).astype(np.float32)     # [128,3]

    xb = x.reshape(B, C, N)
    in_maps = []
    for b in range(B):
        in_maps.append({
            "x": np.ascontiguousarray(xb[b]),
            "xb": np.ascontiguousarray(xb[b].astype(bf)),
            "wpack": wpack, "bpack": bpack,
        })
    return in_maps


def run(inputs, trace=False, **kw):
    from concourse.bass_utils import run_bass_kernel_spmd

    if "nc" not in _CACHE:
        _CACHE["nc"] = build_nc()
    nc = _CACHE["nc"]
    in_maps = host_prep(inputs)
    try:
        res = run_bass_kernel_spmd(nc, in_maps, core_ids=list(range(NCORES)),
                                   trace=trace, **kw)
    except Exception:
        # transient device wedge (e.g. NRT_EXEC_UNIT_UNRECOVERABLE from an
        # earlier crashed process) -- retry once
        res = run_bass_kernel_spmd(nc, in_maps, core_ids=list(range(NCORES)),
                                   trace=trace, **kw)
    y = np.stack([np.asarray(res.results[b]["y"]) for b in range(B)])
    y = y.reshape(B, C, W, H).astype(np.float32)
    return y, res


def kernel(**inputs) -> np.ndarray:
    y, _ = run(inputs)
    return y


# revision 27
# speedup vs baseline: 1.0108x; 1.0108x over previous
"""ConvSelfAttention Trainium2 kernel.

Reference computation (per batch b, with x flattened to [C=128, N=4096]):
    q = wq @ x + bq        [64, N]   (scaled by 1/sqrt(128), folded into wq/bq)
    k = wk @ x + bk        [64, N]
    v = wv @ x + bv        [64, N]
    s[i,j] = sum_o q[o,i] k[o,j]
    p = softmax_j(s)
    out[o,i] = sum_j v[o,j] p[i,j]
    y = gamma * (wo @ out + bo) + x

Mapping (one batch per NeuronCore, 8 cores):
  - scores are built TRANSPOSED: sT[j,i] = sum_o k[o,j] q[o,i], j-tile (128) on
    partitions, i-block (512) on free dim; exp runs on ScalarE over 3-bank PSUM
    groups.
  - QK has K=64 (head dim), so q/k are kept DUPLICATED in both partition
    halves (duplication is free: the projection weight matrix is duplicated on
    the host) and consecutive j-tiles run CONCURRENTLY in the PE array via
    row tile_position (0,0)/(64,0).
  - PV uses the exp output pT as the STATIONARY operand (M=128 i-columns, FWL
    loads bf16 weights 2/cycle) streaming the ones-augmented V^T (N=65):
    psum[i, 0:64] accumulates attention output (transposed), psum[i, 64] the
    softmax denominator -- per-partition, so normalization is a plain
    tensor_scalar multiply and the reciprocal is a [128, 4] op.
  - the normalized [i, o] tile is PE-transposed back to [o, i] for the output
    projection; v's bias folds into the constant gamma*(wo@bv+bo) because
    sum_j p = 1; gamma folds into wo/bo on the host.

All matmuls run in bf16 (fp32 PSUM accumulation); exp runs on ScalarE in fp32.
"""

import sys

import numpy as np

try:
    import concourse  # noqa: F401
except ImportError:  # pragma: no cover
    sys.path.insert(0, "/opt/trn_rl_repo")

import ml_dtypes

B, C, CO, N = 8, 128, 64, 4096
W = H = 64
NCORES = 8
IBLK = 512          # query columns per i-block
NSUB = IBLK // 128  # 4 i-subtiles per i-block (PV stationary operand width)
NJT = N // 128      # 32 j-tiles of 128 keys
NIB = N // IBLK     # 8 i-blocks
JGRP = 3            # j-tiles per exp group (3 PSUM banks)

_CACHE = {}


def _split_multiwaits(nc):
    """Workaround for the pinned walrus: it accepts at most ONE semaphore wait
    per instruction (setupSyncWait: "Too many sync wait commands").  Hoist all
    but the last wait of any instruction onto single-wait NoOps inserted just
    before it in the same engine's stream — semantically identical (the engine
    blocks on each wait in turn before issuing the instruction)."""
    from concourse import mybir

    nsplit = 0
    for fn in nc.m.functions:
        for bb in fn.blocks:
            out = []
            for inst in bb.instructions:
                si = inst.sync_info
                if si is not None and si.on_wait is not None and len(si.on_wait) > 1:
                    waits = list(si.on_wait)
                    for i, w in enumerate(waits[:-1]):
                        out.append(mybir.InstNoOp(
                            name=f"{inst.name}-sw{i}",
                            engine=inst.engine,
                            sync_info=mybir.SyncInfo(on_wait=[w], on_update=[]),
                            bass_nofuse=True,
                        ))
                        nsplit += 1
                    si.on_wait = [waits[-1]]
                    inst.sync_info = si
                out.append(inst)
            bb.instructions = out
    return nsplit


def build_nc(debug=False, nib=NIB, use_bacc=False, split=True):
    from concourse import bacc, mybir
    import concourse.bass as bass
    import concourse.tile as tile
    from concourse.masks import make_identity

    f32 = mybir.dt.float32
    bf16 = mybir.dt.bfloat16
    Alu = mybir.AluOpType
    Act = mybir.ActivationFunctionType

    if use_bacc:
        nc = bacc.Bacc(None, target_bir_lowering=False, debug=debug)
    else:
        nc = bass.Bass()

    x_d = nc.dram_tensor("x", [C, N], f32, kind="ExternalInput")
    xb_d = nc.dram_tensor("xb", [C, N], bf16, kind="ExternalInput")  # host cast
    # packed bf16 weights: [wqT(128) | wkT(128) | wvT(64) | woT(128, rows 0:64)]
    wpack_d = nc.dram_tensor("wpack", [C, 448], bf16, kind="ExternalInput")
    # packed f32 scalars: [bq | bk | gbo]
    bpack_d = nc.dram_tensor("bpack", [C, 3], f32, kind="ExternalInput")
    y_d = nc.dram_tensor("y", [C, N], f32, kind="ExternalOutput")

    with tile.TileContext(nc) as tc:
        with (
            tc.tile_pool(name="consts", bufs=1) as consts,
            tc.tile_pool(name="big", bufs=1) as big,
            tc.tile_pool(name="pt", bufs=6) as pt_pool,
            tc.tile_pool(name="epi", bufs=2) as epi,
        ):
            # ---- x chunk 0 DMA first (critical path), then packed weights ----
            x_sb = big.tile([C, N], f32)
            x_bf = big.tile([C, N], bf16)
            nc.sync.dma_start(x_bf[:, 0:512], xb_d[:, 0:512])

            wpack = consts.tile([C, 448], bf16)
            nc.gpsimd.dma_start(wpack, wpack_d[:, :])
            bpack = consts.tile([C, 3], f32)
            nc.gpsimd.dma_start(bpack, bpack_d[:, :])
            wqT = wpack[:, 0:128]
            wkT = wpack[:, 128:256]
            wvT = wpack[:, 256:320]
            woT = wpack[0:CO, 320:448]
            bq_s = bpack[:, 0:1]
            bk_s = bpack[:, 1:2]
            gbo = bpack[:, 2:3]
            ident = consts.tile([C, C], bf16)
            make_identity(nc, ident)

            # warm the exp table set (~2.7us ACT_TABLE_LOAD) during the ramp;
            # memset source so no DMA dependency
            warm = consts.tile([C, 1], f32)
            nc.vector.memset(warm, 0.0)
            nc.scalar.activation(warm, warm, Act.Exp)

            # ---- x load + cast + projections, pipelined in 512-col chunks ----
            q_sb = big.tile([C, N], bf16)
            k_sb = big.tile([C, N], bf16)
            vT = big.tile([C, NJT * (CO + 1)], bf16)  # 32 x [128, 65] tiles
            vT3 = vT.rearrange("p (t e) -> p t e", e=CO + 1)
            nc.vector.memset(vT3[:, :, CO:CO + 1], 1.0)

            with tc.tile_pool(name="setup_ps", bufs=2, space="PSUM") as setup_ps:
                for t in range(N // 512):
                    sl = slice(t * 512, (t + 1) * 512)
                    if t > 0:  # chunk 0 DMA already issued above
                        nc.sync.dma_start(x_bf[:, sl], xb_d[:, sl])
                    # x f32 (residual add) loads independently, off the
                    # projection critical path
                    nc.sync.dma_start(x_sb[:, sl], x_d[:, sl])
                    ps_q = setup_ps.tile([C, 512], f32, tag="proj")
                    nc.tensor.matmul(ps_q, lhsT=wqT, rhs=x_bf[:, sl],
                                     start=True, stop=True)
                    nc.vector.tensor_scalar_add(q_sb[:, sl], ps_q, bq_s)
                    ps_k = setup_ps.tile([C, 512], f32, tag="proj")
                    nc.tensor.matmul(ps_k, lhsT=wkT, rhs=x_bf[:, sl],
                                     start=True, stop=True)
                    nc.vector.tensor_scalar_add(k_sb[:, sl], ps_k, bk_s)
                    ps_v = setup_ps.tile([C, 256], f32, tag="vt")
                    for tt in range(4):
                        nt = t * 4 + tt
                        nc.tensor.matmul(
                            ps_v[:, tt * CO:(tt + 1) * CO],
                            lhsT=x_bf[:, nt * 128:(nt + 1) * 128],
                            rhs=wvT,
                            start=True, stop=True,
                        )
                    nc.vector.tensor_copy(
                        vT3[:, t * 4:(t + 1) * 4, 0:CO],
                        ps_v.rearrange("p (t e) -> p t e", e=CO),
                    )

            # ---- main loop over query blocks ----
            with (
                tc.tile_pool(name="qk_ps", bufs=2, space="PSUM") as qk_ps_pool,
                tc.tile_pool(name="pv_ps", bufs=1, space="PSUM") as pv_ps_pool,
                tc.tile_pool(name="oc_ps", bufs=1, space="PSUM") as oc_ps_pool,
            ):
                jgroups = [list(range(s, min(s + JGRP, NJT)))
                           for s in range(0, NJT, JGRP)]
                for ib in range(nib):
                    isl = slice(ib * IBLK, (ib + 1) * IBLK)
                    # [i, o|denom] accumulators for 4 i-subtiles in ONE bank
                    ps_pv = pv_ps_pool.tile([C, NSUB * (CO + 1)], f32, tag="pv")
                    for jts in jgroups:
                        glen = len(jts)
                        ps_qk = qk_ps_pool.tile([128, JGRP * 512], f32)
                        for idx, jt in enumerate(jts):
                            half = jt % 2  # alternate row halves -> PE pairs
                            hsl = slice(half * CO, half * CO + CO)
                            nc.tensor.matmul(
                                ps_qk[:, idx * 512:(idx + 1) * 512],
                                lhsT=k_sb[hsl, jt * 128:(jt + 1) * 128],
                                rhs=q_sb[hsl, isl],
                                start=True, stop=True,
                            )
                        pT = pt_pool.tile([128, JGRP * 512], bf16)
                        nc.scalar.activation(pT[:, 0:glen * 512],
                                             ps_qk[:, 0:glen * 512], Act.Exp)
                        for idx, jt in enumerate(jts):
                            for s in range(NSUB):
                                # start/stop once per BANK: start=True clears
                                # the whole 2KB zero region, so only the very
                                # first MM starts and the very last stops; the
                                # other sub-slices rely on per-element
                                # has_written (cleared by the start) for
                                # overwrite-then-accumulate semantics.
                                nc.tensor.matmul(
                                    ps_pv[:, s * (CO + 1):(s + 1) * (CO + 1)],
                                    lhsT=pT[:, idx * 512 + s * 128:
                                            idx * 512 + (s + 1) * 128],
                                    rhs=vT3[:, jt, :],
                                    start=(jt == 0 and s == 0),
                                    stop=(jt == NJT - 1 and s == NSUB - 1),
                                )

                    # epilogue: per-partition normalize, transpose, project
                    rden = epi.tile([C, NSUB], f32, tag="rden")
                    pv3 = ps_pv.rearrange("p (s e) -> p s e", e=CO + 1)
                    nc.vector.reciprocal(rden, pv3[:, :, CO])
                    onT = epi.tile([C, NSUB * CO], bf16, tag="onT")
                    for s in range(NSUB):
                        nc.vector.tensor_scalar_mul(
                            onT[:, s * CO:(s + 1) * CO],
                            pv3[:, s, 0:CO],
                            rden[:, s:s + 1],
                        )
                    out_sb = epi.tile([CO, IBLK], bf16, tag="out")
                    for s in range(NSUB):
                        ps_t = oc_ps_pool.tile([CO, 128], bf16, tag="oc")
                        nc.tensor.transpose(ps_t, onT[:, s * CO:(s + 1) * CO],
                                            ident)
                        nc.vector.tensor_copy(out_sb[:, s * 128:(s + 1) * 128],
                                              ps_t)
                    ps_oc = oc_ps_pool.tile([C, IBLK], f32, tag="oc")
                    nc.tensor.matmul(ps_oc, lhsT=woT, rhs=out_sb,
                                     start=True, stop=True)
                    y2 = epi.tile([C, IBLK], f32, tag="y2")
                    nc.vector.scalar_tensor_tensor(
                        out=y2, in0=ps_oc, scalar=gbo, in1=x_sb[:, isl],
                        op0=Alu.add, op1=Alu.add,
                    )
                    nc.sync.dma_start(y_d[:, isl], y2)

    if split:
        _split_multiwaits(nc)
    return nc


def host_prep(inputs):
    """Fold scales/transposes on the host; returns the 8 per-core input maps."""
    x = np.ascontiguousarray(np.asarray(inputs["x"], dtype=np.float32))
    wq = np.asarray(inputs["wq"], dtype=np.float32)
    bq = np.asarray(inputs["bq"], dtype=np.float32)
    wk = np.asarray(inputs["wk"], dtype=np.float32)
    bk = np.asarray(inputs["bk"], dtype=np.float32)
    wv = np.asarray(inputs["wv"], dtype=np.float32)
    bv = np.asarray(inputs["bv"], dtype=np.float32)
    wo = np.asarray(inputs["wo"], dtype=np.float32)
    bo = np.asarray(inputs["bo"], dtype=np.float32)
    gamma = float(np.asarray(inputs["gamma"]).reshape(-1)[0])

    s = 1.0 / np.sqrt(np.float32(C))
    bf = ml_dtypes.bfloat16
    wqTs = wq.T * s                                                    # [128,64]
    wqT = np.concatenate([wqTs, wqTs], axis=1)                         # [128,128]
    wkT = np.concatenate([wk.T, wk.T], axis=1)                         # [128,128]
    wvT = wv.T                                                         # [128,64]
    woT_pad = np.zeros((C, C), np.float32)
    woT_pad[:CO, :] = gamma * wo.T                                     # rows 0:64
    wpack = np.concatenate([wqT, wkT, wvT, woT_pad], axis=1).astype(bf)
    bq_s = np.concatenate([bq * s, bq * s])
    bk_s = np.concatenate([bk, bk])
    gbo = gamma * (wo @ bv + bo)
    bpack = np.stack([bq_s, bk_s, gbo], axis=1).astype(np.float32)     # [128,3]

    xb = x.reshape(B, C, N)
    in_maps = []
    for b in range(B):
        in_maps.append({
            "x": np.ascontiguousarray(xb[b]),
            "xb": np.ascontiguousarray(xb[b].astype(bf)),
            "wpack": wpack, "bpack": bpack,
        })
    return in_maps


def run(inputs, trace=False, **kw):
    from concourse.bass_utils import run_bass_kernel_spmd

    if "nc" not in _CACHE:
        _CACHE["nc"] = build_nc()
    nc = _CACHE["nc"]
    in_maps = host_prep(inputs)
    try:
        res = run_bass_kernel_spmd(nc, in_maps, core_ids=list(range(NCORES)),
                                   trace=trace, **kw)
    except Exception:
        # transient device wedge (e.g. NRT_EXEC_UNIT_UNRECOVERABLE from an
        # earlier crashed process) -- retry once
        res = run_bass_kernel_spmd(nc, in_maps, core_ids=list(range(NCORES)),
                                   trace=trace, **kw)
    y = np.stack([np.asarray(res.results[b]["y"]) for b in range(B)])
    y = y.reshape(B, C, W, H).astype(np.float32)
    return y, res


def kernel(**inputs) -> np.ndarray:
    y, _ = run(inputs)
    return y
